# revision 1
# baseline (speedup 1.0000x reference)
"""Single-head full-attention layer on 8 Trainium2 NeuronCores.

reference:
    q = seq @ Wq; k = seq @ Wk; v = seq @ Wv          # [B,S,D], D=1024
    scores = q @ k.T / sqrt(D)                        # [B,S,S]
    out = seq + softmax(scores) @ v * mask            # [B,S,D]

Sharding: 8 cores = 4 batches x 2 query-halves. Each core computes K/V
for its whole batch (duplicated within the pair) and Q for its half.
All tensors are kept transposed on device ([d] or [key] on partitions,
queries on the free axis) so every matmul is a natural lhsT.T @ rhs:

    QT = WqT-chunks.T @ seqT       [d, q]     (lhsT=Wq chunk)
    KT = WkT-chunks.T @ seqT       [d, key]
    V  = seqT-chunks.T @ Wv        [key, d]
    ST = KT-chunks.T @ QT          [key, q]
    AT = exp(ST / 32)              bf16, unnormalized (scores ~ N(0,1))
    colsum[q] = ones.T @ AT        (PE reduction over keys)
    OT = V-chunks.T @ AT           [d, q]
    out = OT * (1/colsum) + seqT_half_f32    (mask folded into Wv on host)

The SPMD program is identical on all cores: the host feeds each core
seq[b].T with its own query-half's columns FIRST (key order permutation
is invariant under softmax + attn@V).
"""

import numpy as np
import ml_dtypes

import concourse.bass as bass
import concourse.mybir as mybir
import concourse.tile as tile
from concourse import bacc, bass_utils

B, S, D = 4, 2048, 1024
N_CORES = 8
SH = S // 2          # queries per core
PD = 128             # partition dim
KD = D // PD         # 8 chunks over d
KC = S // PD         # 16 chunks over keys
NT = 512             # matmul free-dim tile (one PSUM bank of fp32)
BF = mybir.dt.bfloat16
F32 = mybir.dt.float32
EXP_SCALE = 1.0 / 32.0   # 1/sqrt(D)

_BF16 = ml_dtypes.bfloat16


def _build_kernel(tc):
    nc = tc.nc
    seqT = nc.dram_tensor("seqT", [D, S], BF, kind="ExternalInput").ap()
    wq = nc.dram_tensor("wq", [D, D], BF, kind="ExternalInput").ap()
    wk = nc.dram_tensor("wk", [D, D], BF, kind="ExternalInput").ap()
    wv = nc.dram_tensor("wv", [D, D], BF, kind="ExternalInput").ap()
    seqTh = nc.dram_tensor("seqTh", [D, SH], F32, kind="ExternalInput").ap()
    outT = nc.dram_tensor("outT", [D, SH], F32, kind="ExternalOutput").ap()

    Exp = mybir.ActivationFunctionType.Exp

    with (
        tc.tile_pool(name="p_seq", bufs=1) as p_seq,
        tc.tile_pool(name="p_w", bufs=2) as p_w,
        tc.tile_pool(name="p_qt", bufs=1) as p_qt,
        tc.tile_pool(name="p_kt", bufs=1) as p_kt,
        tc.tile_pool(name="p_v", bufs=1) as p_v,
        tc.tile_pool(name="p_at", bufs=1) as p_at,
        tc.tile_pool(name="p_sh", bufs=2) as p_sh,
        tc.tile_pool(name="p_o", bufs=2) as p_o,
        tc.tile_pool(name="p_msc", bufs=1) as p_msc,
        tc.tile_pool(name="p_mm", bufs=6, space="PSUM") as p_mm,
        tc.tile_pool(name="p_cs", bufs=1, space="PSUM") as p_cs,
    ):
        # ---- resident inputs ------------------------------------------------
        seq_sb = []
        for i in range(KD):
            t = p_seq.tile([PD, S], BF, tag=f"s{i}", name=f"seq{i}")
            nc.sync.dma_start(t[:], seqT[i * PD:(i + 1) * PD, :])
            seq_sb.append(t)

        def load_w(w_dram, label):
            chunks = []
            for i in range(KD):
                t = p_w.tile([PD, D], BF, tag=f"w{i}", name=f"{label}{i}")
                nc.sync.dma_start(t[:], w_dram[i * PD:(i + 1) * PD, :])
                chunks.append(t)
            return chunks

        # ---- QT = (seq @ Wq).T for this core's query half -------------------
        wq_sb = load_w(wq, "wq")
        qt_sb = [p_qt.tile([PD, SH], BF, tag=f"q{m}", name=f"qt{m}") for m in range(KD)]
        for m in range(KD):
            for n in range(SH // NT):
                ps = p_mm.tile([PD, NT], F32, tag="mm", name=f"ps_q{m}_{n}")
                for k in range(KD):
                    nc.tensor.matmul(
                        ps[:],
                        wq_sb[k][:, m * PD:(m + 1) * PD],
                        seq_sb[k][:, n * NT:(n + 1) * NT],
                        start=(k == 0),
                        stop=(k == KD - 1),
                    )
                nc.vector.tensor_copy(qt_sb[m][:, n * NT:(n + 1) * NT], ps[:])

        # ---- KT = (seq @ Wk).T over all keys --------------------------------
        wk_sb = load_w(wk, "wk")
        kt_sb = [p_kt.tile([PD, S], BF, tag=f"k{m}", name=f"kt{m}") for m in range(KD)]
        for m in range(KD):
            for n in range(S // NT):
                ps = p_mm.tile([PD, NT], F32, tag="mm", name=f"ps_k{m}_{n}")
                for k in range(KD):
                    nc.tensor.matmul(
                        ps[:],
                        wk_sb[k][:, m * PD:(m + 1) * PD],
                        seq_sb[k][:, n * NT:(n + 1) * NT],
                        start=(k == 0),
                        stop=(k == KD - 1),
                    )
                nc.vector.tensor_copy(kt_sb[m][:, n * NT:(n + 1) * NT], ps[:])

        # ---- V = seq @ (Wv * mask), natural [key, d] layout -----------------
        wv_sb = load_w(wv, "wv")
        v_sb = [p_v.tile([PD, D], BF, tag=f"v{m}", name=f"v{m}") for m in range(KC)]
        for m in range(KC):
            for n in range(D // NT):
                ps = p_mm.tile([PD, NT], F32, tag="mm", name=f"ps_v{m}_{n}")
                for k in range(KD):
                    nc.tensor.matmul(
                        ps[:],
                        seq_sb[k][:, m * PD:(m + 1) * PD],
                        wv_sb[k][:, n * NT:(n + 1) * NT],
                        start=(k == 0),
                        stop=(k == KD - 1),
                    )
                nc.vector.tensor_copy(v_sb[m][:, n * NT:(n + 1) * NT], ps[:])

        # ---- scoresT -> exp -> colsum ---------------------------------------
        ones_sb = p_msc.tile([PD, 1], BF, tag="ones", name="ones")
        nc.vector.memset(ones_sb[:], 1.0)
        cs_ps = p_cs.tile([1, SH], F32, tag="cs", name="cs")
        at_sb = [p_at.tile([PD, SH], BF, tag=f"a{m}", name=f"at{m}") for m in range(KC)]

        def colsum_mm(m):
            for n in range(SH // NT):
                nc.tensor.matmul(
                    cs_ps[:, n * NT:(n + 1) * NT],
                    ones_sb[:],
                    at_sb[m][:, n * NT:(n + 1) * NT],
                    start=(m == 0),
                    stop=(m == KC - 1),
                )

        for m in range(KC):
            for n in range(SH // NT):
                ps = p_mm.tile([PD, NT], F32, tag="mm", name=f"ps_s{m}_{n}")
                for k in range(KD):
                    nc.tensor.matmul(
                        ps[:],
                        kt_sb[k][:, m * PD:(m + 1) * PD],
                        qt_sb[k][:, n * NT:(n + 1) * NT],
                        start=(k == 0),
                        stop=(k == KD - 1),
                    )
                nc.scalar.activation(
                    at_sb[m][:, n * NT:(n + 1) * NT], ps[:], Exp, scale=EXP_SCALE
                )
            # one chunk late so the PE never waits on ACT's exp
            if m > 0:
                colsum_mm(m - 1)
        colsum_mm(KC - 1)

        # ---- 1/colsum, broadcast across partitions --------------------------
        recip_sb = p_msc.tile([1, SH], F32, tag="recip", name="recip")
        nc.vector.reciprocal(recip_sb[:], cs_ps[:])
        bc_sb = p_msc.tile([PD, SH], F32, tag="bc", name="bc")
        nc.gpsimd.partition_broadcast(bc_sb[:], recip_sb[:])

        # ---- OT = V.T @ AT, normalize, add residual, store ------------------
        for m in range(KD):
            sh_t = p_sh.tile([PD, SH], F32, tag="sh", name=f"sh{m}")
            nc.sync.dma_start(sh_t[:], seqTh[m * PD:(m + 1) * PD, :])
            o_t = p_o.tile([PD, SH], F32, tag="o", name=f"o{m}")
            for n in range(SH // NT):
                ps = p_mm.tile([PD, NT], F32, tag="mm", name=f"ps_o{m}_{n}")
                for k in range(KC):
                    nc.tensor.matmul(
                        ps[:],
                        v_sb[k][:, m * PD:(m + 1) * PD],
                        at_sb[k][:, n * NT:(n + 1) * NT],
                        start=(k == 0),
                        stop=(k == KC - 1),
                    )
                nc.vector.tensor_mul(
                    o_t[:, n * NT:(n + 1) * NT], ps[:], bc_sb[:, n * NT:(n + 1) * NT]
                )
            nc.vector.tensor_add(o_t[:], o_t[:], sh_t[:])
            nc.sync.dma_start(outT[m * PD:(m + 1) * PD, :], o_t[:])


_NC_CACHE = None


def _get_nc():
    global _NC_CACHE
    if _NC_CACHE is None:
        nc = bacc.Bacc(
            "TRN2", target_bir_lowering=False, debug=False, num_devices=N_CORES
        )
        with tile.TileContext(nc) as tc:
            _build_kernel(tc)
        nc.compile()
        _NC_CACHE = nc
    return _NC_CACHE


def _prep_in_maps(seq, Wq, Wk, Wv, mask):
    seq = np.asarray(seq, dtype=np.float32)
    wq_bf = np.asarray(Wq, dtype=np.float32).astype(_BF16)
    wk_bf = np.asarray(Wk, dtype=np.float32).astype(_BF16)
    wvm_bf = (np.asarray(Wv, dtype=np.float32)
              * np.asarray(mask, dtype=np.float32)[None, :]).astype(_BF16)
    in_maps = []
    for c in range(N_CORES):
        b, h = divmod(c, 2)
        seqT_nat = seq[b].T  # [D, S]
        if h == 0:
            seqT_core = seqT_nat
        else:
            seqT_core = np.concatenate([seqT_nat[:, SH:], seqT_nat[:, :SH]], axis=1)
        in_maps.append({
            "seqT": np.ascontiguousarray(seqT_core).astype(_BF16),
            "wq": wq_bf,
            "wk": wk_bf,
            "wv": wvm_bf,
            "seqTh": np.ascontiguousarray(seqT_core[:, :SH], dtype=np.float32),
        })
    return in_maps


def _run(seq, Wq, Wk, Wv, mask, trace=False, **run_kwargs):
    nc = _get_nc()
    in_maps = _prep_in_maps(seq, Wq, Wk, Wv, mask)
    res = bass_utils.run_bass_kernel_spmd(
        nc, in_maps, core_ids=list(range(N_CORES)), trace=trace, **run_kwargs
    )
    out = np.empty((B, S, D), dtype=np.float32)
    for c in range(N_CORES):
        b, h = divmod(c, 2)
        out[b, h * SH:(h + 1) * SH, :] = res.results[c]["outT"].T
    return out, res


def kernel(seq, Wq, Wk, Wv, mask):
    out, _ = _run(seq, Wq, Wk, Wv, mask)
    return out


# revision 2
# speedup vs baseline: 1.1676x; 1.1676x over previous
"""Single-head full-attention layer on 8 Trainium2 NeuronCores.

reference:
    q = seq @ Wq; k = seq @ Wk; v = seq @ Wv          # [B,S,D], D=1024
    scores = q @ k.T / sqrt(D)                        # [B,S,S]
    out = seq + softmax(scores) @ v * mask            # [B,S,D]

Sharding: 8 cores = 4 batches x 2 sequence-halves. Each core:
  - computes Q for its own 1024 queries,
  - computes K^T/V for its own 1024 keys only,
  - exchanges K^T/V halves with its pair partner via 2-core AllGathers
    (K exchanged early so its latency hides under the Q projection),
  - runs softmax(QK^T)V + mask + residual for its query half.

All tensors are kept transposed on device ([d] or [key] on partitions,
queries on the free axis) so every matmul is a natural lhsT.T @ rhs:

    KT_own = Wk-chunks.T @ seqT_own   [d, key_own]   -> AllGather -> KT
    V_own  = seqT-chunks.T @ Wv       [key_own, d]   -> AllGather -> V
    QT     = Wq-chunks.T @ seqT_own   [d, q]
    ST     = KT-chunks.T @ QT         [key, q]
    AT     = exp(ST / 32)             bf16, unnormalized (scores ~ N(0,1))
    colsum[q] = ones.T @ AT           (PE reduction over keys)
    OT     = V-chunks.T @ AT          [d, q]
    out    = OT * (1/colsum) + seqT_half_f32   (mask folded into Wv on host)

The SPMD program is identical on all cores: the host feeds each core the
transposed bf16 slice of seq for its own half; the AllGather delivers
keys in global order for everyone.
"""

import numpy as np
import ml_dtypes

import concourse.bass as bass
import concourse.mybir as mybir
import concourse.tile as tile
from concourse import bacc, bass_utils

B, S, D = 4, 2048, 1024
N_CORES = 8
SH = S // 2          # queries / own keys per core
PD = 128             # partition dim
KD = D // PD         # 8 chunks over d
KH = SH // PD        # 8 chunks over own keys
KC = S // PD         # 16 chunks over all keys
NT = 512             # matmul free-dim tile (one PSUM bank of fp32)
BF = mybir.dt.bfloat16
F32 = mybir.dt.float32
EXP_SCALE = 1.0 / 32.0   # 1/sqrt(D)

_BF16 = ml_dtypes.bfloat16


def _build_kernel(tc):
    nc = tc.nc
    seqT = nc.dram_tensor("seqT", [D, SH], BF, kind="ExternalInput").ap()
    wq = nc.dram_tensor("wq", [D, D], BF, kind="ExternalInput").ap()
    wk = nc.dram_tensor("wk", [D, D], BF, kind="ExternalInput").ap()
    wv = nc.dram_tensor("wv", [D, D], BF, kind="ExternalInput").ap()
    seqTh = nc.dram_tensor("seqTh", [D, SH], F32, kind="ExternalInput").ap()
    outT = nc.dram_tensor("outT", [D, SH], F32, kind="ExternalOutput").ap()

    Exp = mybir.ActivationFunctionType.Exp

    with (
        tc.tile_pool(name="p_seq", bufs=1) as p_seq,
        tc.tile_pool(name="p_w", bufs=2) as p_w,
        tc.tile_pool(name="p_own", bufs=1) as p_own,
        tc.tile_pool(name="p_qt", bufs=1) as p_qt,
        tc.tile_pool(name="p_kt", bufs=1) as p_kt,
        tc.tile_pool(name="p_v", bufs=1) as p_v,
        tc.tile_pool(name="p_at", bufs=1) as p_at,
        tc.tile_pool(name="p_sh", bufs=2) as p_sh,
        tc.tile_pool(name="p_o", bufs=2) as p_o,
        tc.tile_pool(name="p_msc", bufs=1) as p_msc,
        tc.tile_pool(name="p_dram", bufs=1, space="DRAM") as p_dram,
        tc.tile_pool(name="p_mm", bufs=6, space="PSUM") as p_mm,
        tc.tile_pool(name="p_cs", bufs=1, space="PSUM") as p_cs,
    ):
        # ---- resident inputs ------------------------------------------------
        seq_sb = []
        for i in range(KD):
            t = p_seq.tile([PD, SH], BF, tag=f"s{i}", name=f"seq{i}")
            nc.sync.dma_start(t[:], seqT[i * PD:(i + 1) * PD, :])
            seq_sb.append(t)

        def load_w(w_dram, label):
            chunks = []
            for i in range(KD):
                t = p_w.tile([PD, D], BF, tag=f"w{i}", name=f"{label}{i}")
                nc.sync.dma_start(t[:], w_dram[i * PD:(i + 1) * PD, :])
                chunks.append(t)
            return chunks

        # collective bounce buffers (DRAM, Local)
        ib_kt = p_dram.tile([D, SH], BF, tag="ibk", name="ib_kt")
        ob_kt = p_dram.tile([2, D, SH], BF, tag="obk", name="ob_kt")
        ib_v = p_dram.tile([SH, D], BF, tag="ibv", name="ib_v")
        ob_v = p_dram.tile([2, SH, D], BF, tag="obv", name="ob_v")

        # ---- KT_own = (seq_own @ Wk).T, bounce out, AllGather ---------------
        wk_sb = load_w(wk, "wk")
        kto_sb = []
        for m in range(KD):
            t = p_own.tile([PD, SH], BF, tag=f"x{m}", name=f"kto{m}")
            kto_sb.append(t)
        for m in range(KD):
            for n in range(SH // NT):
                ps = p_mm.tile([PD, NT], F32, tag="mm", name=f"ps_k{m}_{n}")
                for k in range(KD):
                    nc.tensor.matmul(
                        ps[:],
                        wk_sb[k][:, m * PD:(m + 1) * PD],
                        seq_sb[k][:, n * NT:(n + 1) * NT],
                        start=(k == 0),
                        stop=(k == KD - 1),
                    )
                nc.vector.tensor_copy(kto_sb[m][:, n * NT:(n + 1) * NT], ps[:])
            nc.sync.dma_start(ib_kt[m * PD:(m + 1) * PD, :], kto_sb[m][:])
        nc.gpsimd.collective_compute(
            "AllGather",
            mybir.AluOpType.bypass,
            replica_groups=[[0, 1], [2, 3], [4, 5], [6, 7]],
            ins=[ib_kt.opt()],
            outs=[ob_kt.opt()],
        )

        # ---- V_own = seq_own @ (Wv * mask), bounce out, AllGather -----------
        wv_sb = load_w(wv, "wv")
        vo_sb = []
        for m in range(KH):
            t = p_own.tile([PD, D], BF, tag=f"x{m}", name=f"vo{m}")
            vo_sb.append(t)
        for m in range(KH):
            for n in range(D // NT):
                ps = p_mm.tile([PD, NT], F32, tag="mm", name=f"ps_v{m}_{n}")
                for k in range(KD):
                    nc.tensor.matmul(
                        ps[:],
                        seq_sb[k][:, m * PD:(m + 1) * PD],
                        wv_sb[k][:, n * NT:(n + 1) * NT],
                        start=(k == 0),
                        stop=(k == KD - 1),
                    )
                nc.vector.tensor_copy(vo_sb[m][:, n * NT:(n + 1) * NT], ps[:])
            nc.sync.dma_start(ib_v[m * PD:(m + 1) * PD, :], vo_sb[m][:])
        nc.gpsimd.collective_compute(
            "AllGather",
            mybir.AluOpType.bypass,
            replica_groups=[[0, 1], [2, 3], [4, 5], [6, 7]],
            ins=[ib_v.opt()],
            outs=[ob_v.opt()],
        )

        # ---- QT = (seq_own @ Wq).T (overlaps the collectives) ---------------
        wq_sb = load_w(wq, "wq")
        qt_sb = [p_qt.tile([PD, SH], BF, tag=f"q{m}", name=f"qt{m}") for m in range(KD)]
        for m in range(KD):
            for n in range(SH // NT):
                ps = p_mm.tile([PD, NT], F32, tag="mm", name=f"ps_q{m}_{n}")
                for k in range(KD):
                    nc.tensor.matmul(
                        ps[:],
                        wq_sb[k][:, m * PD:(m + 1) * PD],
                        seq_sb[k][:, n * NT:(n + 1) * NT],
                        start=(k == 0),
                        stop=(k == KD - 1),
                    )
                nc.vector.tensor_copy(qt_sb[m][:, n * NT:(n + 1) * NT], ps[:])

        # ---- gather exchanged KT / V into SBUF ------------------------------
        kt_sb = [p_kt.tile([PD, S], BF, tag=f"k{m}", name=f"kt{m}") for m in range(KD)]
        for m in range(KD):
            for r in range(2):
                nc.sync.dma_start(
                    kt_sb[m][:, r * SH:(r + 1) * SH],
                    ob_kt[r, m * PD:(m + 1) * PD, :],
                )
        v_sb = [p_v.tile([PD, D], BF, tag=f"v{m}", name=f"v{m}") for m in range(KC)]
        for m in range(KC):
            r, mm_ = divmod(m, KH)
            nc.sync.dma_start(v_sb[m][:], ob_v[r, mm_ * PD:(mm_ + 1) * PD, :])

        # ---- scoresT -> exp -> colsum ---------------------------------------
        ones_sb = p_msc.tile([PD, 1], BF, tag="ones", name="ones")
        nc.vector.memset(ones_sb[:], 1.0)
        cs_ps = p_cs.tile([1, SH], F32, tag="cs", name="cs")
        at_sb = [p_at.tile([PD, SH], BF, tag=f"a{m}", name=f"at{m}") for m in range(KC)]

        def colsum_mm(m):
            for n in range(SH // NT):
                nc.tensor.matmul(
                    cs_ps[:, n * NT:(n + 1) * NT],
                    ones_sb[:],
                    at_sb[m][:, n * NT:(n + 1) * NT],
                    start=(m == 0),
                    stop=(m == KC - 1),
                )

        for m in range(KC):
            for n in range(SH // NT):
                ps = p_mm.tile([PD, NT], F32, tag="mm", name=f"ps_s{m}_{n}")
                for k in range(KD):
                    nc.tensor.matmul(
                        ps[:],
                        kt_sb[k][:, m * PD:(m + 1) * PD],
                        qt_sb[k][:, n * NT:(n + 1) * NT],
                        start=(k == 0),
                        stop=(k == KD - 1),
                    )
                nc.scalar.activation(
                    at_sb[m][:, n * NT:(n + 1) * NT], ps[:], Exp, scale=EXP_SCALE
                )
            # one chunk late so the PE never waits on ACT's exp
            if m > 0:
                colsum_mm(m - 1)
        colsum_mm(KC - 1)

        # ---- 1/colsum, broadcast across partitions --------------------------
        recip_sb = p_msc.tile([1, SH], F32, tag="recip", name="recip")
        nc.vector.reciprocal(recip_sb[:], cs_ps[:])
        bc_sb = p_msc.tile([PD, SH], F32, tag="bc", name="bc")
        nc.gpsimd.partition_broadcast(bc_sb[:], recip_sb[:])

        # ---- OT = V.T @ AT, normalize, add residual, store ------------------
        for m in range(KD):
            sh_t = p_sh.tile([PD, SH], F32, tag="sh", name=f"sh{m}")
            nc.sync.dma_start(sh_t[:], seqTh[m * PD:(m + 1) * PD, :])
            o_t = p_o.tile([PD, SH], F32, tag="o", name=f"o{m}")
            for n in range(SH // NT):
                ps = p_mm.tile([PD, NT], F32, tag="mm", name=f"ps_o{m}_{n}")
                for k in range(KC):
                    nc.tensor.matmul(
                        ps[:],
                        v_sb[k][:, m * PD:(m + 1) * PD],
                        at_sb[k][:, n * NT:(n + 1) * NT],
                        start=(k == 0),
                        stop=(k == KC - 1),
                    )
                nc.vector.tensor_mul(
                    o_t[:, n * NT:(n + 1) * NT], ps[:], bc_sb[:, n * NT:(n + 1) * NT]
                )
            nc.vector.tensor_add(o_t[:], o_t[:], sh_t[:])
            nc.sync.dma_start(outT[m * PD:(m + 1) * PD, :], o_t[:])


_NC_CACHE = None


def _get_nc():
    global _NC_CACHE
    if _NC_CACHE is None:
        nc = bacc.Bacc(
            "TRN2", target_bir_lowering=False, debug=False, num_devices=N_CORES
        )
        with tile.TileContext(nc) as tc:
            _build_kernel(tc)
        nc.compile()
        _NC_CACHE = nc
    return _NC_CACHE


def _prep_in_maps(seq, Wq, Wk, Wv, mask):
    seq = np.asarray(seq, dtype=np.float32)
    wq_bf = np.asarray(Wq, dtype=np.float32).astype(_BF16)
    wk_bf = np.asarray(Wk, dtype=np.float32).astype(_BF16)
    wvm_bf = (np.asarray(Wv, dtype=np.float32)
              * np.asarray(mask, dtype=np.float32)[None, :]).astype(_BF16)
    in_maps = []
    for c in range(N_CORES):
        b, h = divmod(c, 2)
        seqT_own = np.ascontiguousarray(seq[b, h * SH:(h + 1) * SH, :].T)  # [D, SH]
        in_maps.append({
            "seqT": seqT_own.astype(_BF16),
            "wq": wq_bf,
            "wk": wk_bf,
            "wv": wvm_bf,
            "seqTh": seqT_own,
        })
    return in_maps


def _run(seq, Wq, Wk, Wv, mask, trace=False, **run_kwargs):
    nc = _get_nc()
    in_maps = _prep_in_maps(seq, Wq, Wk, Wv, mask)
    res = bass_utils.run_bass_kernel_spmd(
        nc, in_maps, core_ids=list(range(N_CORES)), trace=trace, **run_kwargs
    )
    out = np.empty((B, S, D), dtype=np.float32)
    for c in range(N_CORES):
        b, h = divmod(c, 2)
        out[b, h * SH:(h + 1) * SH, :] = res.results[c]["outT"].T
    return out, res


def kernel(seq, Wq, Wk, Wv, mask):
    out, _ = _run(seq, Wq, Wk, Wv, mask)
    return out
